# revision 7
# baseline (speedup 1.0000x reference)
"""DecoderRNN (GRU greedy decode + log_softmax) Trainium2 Bass kernel, 8 cores.

Sharding: V (vocab 32000) split 8x4000 across cores for the output projection
and argmax/lse; GRU gate dim (3*1024) split 8x384; h slices AllGathered each
step; tiny per-step stats AllGather resolves global argmax + logsumexp.
Matmuls run in float32r (tf32-like, full PE rate).

Self-contained: only imports numpy + the system concourse package.
"""
import sys

if "/opt/trn_rl_repo" not in sys.path:
    sys.path.insert(0, "/opt/trn_rl_repo")

import numpy as np

import concourse.bacc as bacc
import concourse.bass as bass
import concourse.mybir as mybir
import concourse.tile as tile

H = 1024
V = 32000
B = 32
T = 64
NC = 8          # cores
VC = V // NC    # 4000 vocab rows per core
GC = 3 * H // NC  # 384 gate rows per core
HC = H // NC    # 128 h dims per core
KT = H // 128   # 8 K tiles
NT = 8          # logits N tiles per core: 7x512 + 416
NSZ = [512] * 7 + [VC - 7 * 512]
NOFF = [512 * i for i in range(NT)]

F32 = mybir.dt.float32
F32R = mybir.dt.float32r
I32 = mybir.dt.int32
AF = mybir.ActivationFunctionType
ALU = mybir.AluOpType
BIGIDX = 65536.0


def build_kernel(n_steps=T):
    nc = bacc.Bacc("TRN2", num_devices=NC)

    # ---- I/O ----
    wih_d = nc.dram_tensor("wih", [128, KT, GC], F32R, kind="ExternalInput")
    whh_d = nc.dram_tensor("whh", [128, KT, GC], F32R, kind="ExternalInput")
    gxb_d = nc.dram_tensor("gxb", [1, GC], F32R, kind="ExternalInput")
    ghb_d = nc.dram_tensor("ghb", [1, GC], F32R, kind="ExternalInput")
    wout_d = nc.dram_tensor("wout", [128, KT, VC], F32R, kind="ExternalInput")
    bout_d = nc.dram_tensor("bout", [1, VC], F32R, kind="ExternalInput")
    emb_d = nc.dram_tensor("emb", [V, H], F32, kind="ExternalInput")
    h0t_d = nc.dram_tensor("h0t", [128, KT, B], F32R, kind="ExternalInput")
    h0s_d = nc.dram_tensor("h0s", [B, HC], F32, kind="ExternalInput")
    x0t_d = nc.dram_tensor("x0t", [128, KT, B], F32R, kind="ExternalInput")
    iota_d = nc.dram_tensor("iota", [B, VC // 4], F32, kind="ExternalInput")
    qoff_d = nc.dram_tensor("qoff", [B, 4], F32, kind="ExternalInput")
    ident_d = nc.dram_tensor("ident", [32, 32], F32, kind="ExternalInput")
    ones_d = nc.dram_tensor("ones", [1, B], F32R, kind="ExternalInput")

    logp_d = nc.dram_tensor("logp", [n_steps, B, VC], F32, kind="ExternalOutput")
    hout_d = nc.dram_tensor("hout", [B, HC], F32, kind="ExternalOutput")

    with tile.TileContext(nc, num_cores=NC) as tc:
        with tc.tile_pool(name="wpool", bufs=1) as wp, \
             tc.tile_pool(name="spool", bufs=1) as sp, \
             tc.tile_pool(name="xh", bufs=2) as xh, \
             tc.tile_pool(name="stg", bufs=1) as stg, \
             tc.tile_pool(name="small", bufs=1) as sm, \
             tc.tile_pool(name="pslog", bufs=3, space="PSUM") as pslog, \
             tc.tile_pool(name="psgx", bufs=1, space="PSUM") as psgx, \
             tc.tile_pool(name="psgh", bufs=1, space="PSUM") as psgh, \
             tc.tile_pool(name="pstr", bufs=2, space="PSUM") as pstr, \
             tc.tile_pool(name="psexp", bufs=1, space="PSUM") as psexp, \
             tc.tile_pool(name="dram", bufs=2, space="DRAM") as dram:

            # ---- persistent weights / constants ----
            wih = wp.tile([128, KT, GC], F32R)
            whh = wp.tile([128, KT, GC], F32R)
            wout = wp.tile([128, KT, VC], F32R)
            gxb = wp.tile([1, GC], F32R)
            ghb = wp.tile([1, GC], F32R)
            bout = wp.tile([1, VC], F32R)
            iota = wp.tile([B, VC // 4], F32)
            qoff = wp.tile([B, 4], F32)
            ident = wp.tile([32, 32], F32)
            ones = wp.tile([1, B], F32R)
            for k in range(KT):  # spread the big weight loads across queues
                nc.sync.dma_start(wout[:, k, :], wout_d[:, k, :])
            nc.sync.dma_start(wih[:], wih_d[:])
            nc.sync.dma_start(whh[:], whh_d[:])
            nc.sync.dma_start(gxb[:], gxb_d[:])
            nc.sync.dma_start(ghb[:], ghb_d[:])
            nc.sync.dma_start(bout[:], bout_d[:])
            nc.sync.dma_start(iota[:], iota_d[:])
            nc.sync.dma_start(qoff[:], qoff_d[:])
            nc.sync.dma_start(ident[:], ident_d[:])
            nc.sync.dma_start(ones[:], ones_d[:])

            # initial state
            hT = xh.tile([128, KT, B], F32R, name="hT_init", tag="hT")
            nc.sync.dma_start(hT[:], h0t_d[:])
            xT = xh.tile([128, KT, B], F32R, name="xT_init", tag="xT")
            nc.sync.dma_start(xT[:], x0t_d[:])
            hs_prev = sm.tile([B, HC], F32, name="hs_init", tag="hs", bufs=2)
            nc.sync.dma_start(hs_prev[:], h0s_d[:])

            tok_i = None  # int32 [B,1] token indices (for gather)

            for t in range(n_steps):
                # ---- x for this step ----
                if t > 0:
                    x_sb = sm.tile([B, H], F32, name=f"x_{t}", tag="x", bufs=1)
                    nc.gpsimd.indirect_dma_start(
                        out=x_sb[:], out_offset=None, in_=emb_d[:],
                        in_offset=bass.IndirectOffsetOnAxis(ap=tok_i[:, :1], axis=0),
                    )
                    xT = xh.tile([128, KT, B], F32R, name=f"xT_{t}", tag="xT")
                    for k in range(KT):
                        ptr = pstr.tile([128, B], F32, name=f"trx_{t}_{k}", tag="tr")
                        nc.tensor.transpose(ptr[:], x_sb[:, 128 * k:128 * (k + 1)], ident[:])
                        nc.scalar.activation(xT[:, k, :], ptr[:], AF.Relu)

                # ---- gates ----
                gx = psgx.tile([B, GC], F32, name=f"gx_{t}", tag="gx")
                for k in range(KT):
                    nc.tensor.matmul(gx[:], xT[:, k, :], wih[:, k, :],
                                     start=(k == 0), stop=False)
                nc.tensor.matmul(gx[:], ones[:], gxb[:], start=False, stop=True)
                gh = psgh.tile([B, GC], F32, name=f"gh_{t}", tag="gh")
                for k in range(KT):
                    nc.tensor.matmul(gh[:], hT[:, k, :], whh[:, k, :],
                                     start=(k == 0), stop=False)
                nc.tensor.matmul(gh[:], ones[:], ghb[:], start=False, stop=True)
                ghs = sm.tile([B, GC], F32, name=f"ghs_{t}", tag="ghs")
                nc.scalar.copy(ghs[:], gh[:])

                # ---- GRU elementwise (slices: r=[0:128], z=[128:256], n=[256:384]) ----
                grz = sm.tile([B, 256], F32, name=f"grz_{t}", tag="grz")
                nc.vector.scalar_tensor_tensor(
                    grz[:], gx[:, 0:256], 1.0, ghs[:, 0:256],
                    op0=ALU.mult, op1=ALU.add)
                r = sm.tile([B, HC], F32, name=f"r_{t}", tag="r")
                nc.scalar.activation(r[:], grz[:, 0:128], AF.Sigmoid)
                z = sm.tile([B, HC], F32, name=f"z_{t}", tag="z")
                nc.scalar.activation(z[:], grz[:, 128:256], AF.Sigmoid)
                npre = sm.tile([B, HC], F32, name=f"npre_{t}", tag="npre")
                nc.vector.tensor_tensor(npre[:], r[:], ghs[:, 256:384], op=ALU.mult)
                nc.vector.tensor_tensor(npre[:], npre[:], gx[:, 256:384], op=ALU.add)
                n_t = sm.tile([B, HC], F32, name=f"n_{t}", tag="n")
                nc.scalar.activation(n_t[:], npre[:], AF.Tanh)
                d_t = sm.tile([B, HC], F32, name=f"d_{t}", tag="d")
                nc.vector.tensor_tensor(d_t[:], hs_prev[:], n_t[:], op=ALU.subtract)
                hs = sm.tile([B, HC], F32, name=f"hs_{t}", tag="hs", bufs=2)
                nc.vector.scalar_tensor_tensor(
                    hs[:], z[:], 1.0, d_t[:], op0=ALU.mult, op1=ALU.mult)
                nc.vector.tensor_tensor(hs[:], hs[:], n_t[:], op=ALU.add)
                hs_prev = hs

                # ---- h slice -> transposed -> AllGather -> full hT ----
                ptr_h = pstr.tile([128, B], F32, name=f"trh_{t}", tag="tr")
                nc.tensor.transpose(ptr_h[:], hs[:], ident[:])
                hTs = sm.tile([128, B], F32R, name=f"hTs_{t}", tag="hTs")
                nc.scalar.copy(hTs[:], ptr_h[:])
                agh_i = dram.tile([128, B], F32R, name=f"aghi_{t}", tag="aghi")
                nc.sync.dma_start(agh_i[:], hTs[:])
                agh_o = dram.tile([128 * NC, B], F32R, addr_space="Shared",
                                  name=f"agho_{t}", tag="agho")
                nc.gpsimd.collective_compute(
                    "AllGather", ALU.bypass,
                    replica_groups=[list(range(NC))],
                    ins=[agh_i[:].opt()], outs=[agh_o[:].opt()])
                hT = xh.tile([128, KT, B], F32R, name=f"hT_{t}", tag="hT")
                nc.sync.dma_start(
                    hT[:], agh_o[:].rearrange("(k p) b -> p k b", p=128))

                # ---- logits ----
                stage = stg.tile([B, VC], F32, name=f"stage_{t}", tag="stage")
                for n in range(NT):
                    nsz, noff = NSZ[n], NOFF[n]
                    pl = pslog.tile([B, 512], F32, name=f"pl_{t}_{n}", tag="pl")
                    for k in range(KT):
                        nc.tensor.matmul(pl[:, 0:nsz], hT[:, k, :],
                                         wout[:, k, noff:noff + nsz],
                                         start=(k == 0), stop=False)
                    nc.tensor.matmul(pl[:, 0:nsz], ones[:], bout[0:1, noff:noff + nsz],
                                     start=False, stop=True)
                    nc.scalar.copy(stage[:, noff:noff + nsz], pl[:, 0:nsz])

                # ---- local max (4 quarters), argmax, sumexp ----
                QN, QS = 4, VC // 4
                m4 = sm.tile([B, 4], F32, name=f"m4_{t}", tag="m4")
                for q in range(QN):
                    nc.vector.reduce_max(m4[:, q:q + 1], stage[:, QS * q:QS * (q + 1)],
                                         axis=mybir.AxisListType.X)
                gmax = sm.tile([B, 1], F32, name=f"gmax_{t}", tag="gmax")
                nc.vector.reduce_max(gmax[:], m4[:], axis=mybir.AxisListType.X)
                ngmax = sm.tile([B, 1], F32, name=f"ngmax_{t}", tag="ngmax")
                nc.vector.tensor_scalar_mul(ngmax[:], gmax[:], -1.0)

                if t < n_steps - 1:
                    # local argmax: per-quarter equality*iota sum, then merge
                    eqi = stg.tile([B, QS], F32, name=f"eqi_{t}", tag="outt")
                    c4 = sm.tile([B, 4], F32, name=f"c4_{t}", tag="c4")
                    for q in range(QN):
                        nc.vector.scalar_tensor_tensor(
                            eqi[:], stage[:, QS * q:QS * (q + 1)], m4[:, q:q + 1],
                            iota[:],
                            op0=ALU.is_equal, op1=ALU.mult, accum_out=c4[:, q:q + 1])
                    nc.vector.tensor_tensor(c4[:], c4[:], qoff[:], op=ALU.add)
                    eq4 = sm.tile([B, 4], F32, name=f"eq4_{t}", tag="eq4")
                    nc.vector.tensor_scalar(eq4[:], m4[:], gmax[:, 0:1], None,
                                            op0=ALU.is_equal)
                    key4 = sm.tile([B, 4], F32, name=f"key4_{t}", tag="key4")
                    nc.vector.scalar_tensor_tensor(
                        key4[:], eq4[:], -BIGIDX, c4[:], op0=ALU.mult, op1=ALU.add)
                    lidx = sm.tile([B, 1], F32, name=f"lidx_{t}", tag="lidx")
                    nc.vector.tensor_reduce(lidx[:], key4[:],
                                            axis=mybir.AxisListType.X, op=ALU.min)
                    nc.vector.tensor_scalar_add(lidx[:], lidx[:], BIGIDX)
                else:
                    lidx = gmax  # unused at final step

                # sum(exp(l - gmax)) via 8 tile exps with accumulators
                accs = sm.tile([B, NT], F32, name=f"accs_{t}", tag="accs")
                for n in range(NT):
                    nsz, noff = NSZ[n], NOFF[n]
                    pe_ = psexp.tile([B, 512], F32, name=f"pe_{t}_{n}", tag="pe")
                    nc.scalar.activation(pe_[:, 0:nsz], stage[:, noff:noff + nsz],
                                         AF.Exp, bias=ngmax[:, 0:1],
                                         accum_out=accs[:, n:n + 1])
                ssum = sm.tile([B, 1], F32, name=f"ssum_{t}", tag="ssum")
                nc.vector.reduce_sum(ssum[:], accs[:], axis=mybir.AxisListType.X)

                # ---- pack stats [val|idx|s] -> [1,128] -> AllGather ----
                packT = sm.tile([B, 4], F32, name=f"packT_{t}", tag="packT")
                nc.vector.tensor_copy(packT[:, 0:1], gmax[:])
                nc.vector.tensor_copy(packT[:, 1:2], lidx[:])
                nc.vector.tensor_copy(packT[:, 2:3], ssum[:])
                nc.vector.tensor_copy(packT[:, 3:4], gmax[:])
                ptr_p = pstr.tile([4, B], F32, name=f"trp_{t}", tag="tr")
                nc.tensor.transpose(ptr_p[:], packT[:], ident[:])
                pack = sm.tile([4, B], F32, name=f"pack_{t}", tag="pack")
                nc.scalar.copy(pack[:], ptr_p[:])
                ags_i = dram.tile([1, 128], F32, name=f"agsi_{t}", tag="agsi")
                nc.sync.dma_start(ags_i[:].rearrange("o (p b) -> (o p) b", p=4), pack[:])
                ags_o = dram.tile([NC, 128], F32, addr_space="Shared",
                                  name=f"agso_{t}", tag="agso")
                nc.gpsimd.collective_compute(
                    "AllGather", ALU.bypass,
                    replica_groups=[list(range(NC))],
                    ins=[ags_i[:].opt()], outs=[ags_o[:].opt()])
                ags = sm.tile([NC, 128], F32, name=f"ags_{t}", tag="ags")
                nc.sync.dma_start(ags[:], ags_o[:])

                # ---- combine: global argmax + lse ----
                ptr_v = pstr.tile([B, NC], F32, name=f"trv_{t}", tag="tr")
                nc.tensor.transpose(ptr_v[:], ags[:, 0:B], ident[0:NC, 0:NC])
                v8 = sm.tile([B, NC], F32, name=f"v8_{t}", tag="v8")
                nc.scalar.copy(v8[:], ptr_v[:])
                ptr_s = pstr.tile([B, NC], F32, name=f"trs_{t}", tag="tr")
                nc.tensor.transpose(ptr_s[:], ags[:, 2 * B:3 * B], ident[0:NC, 0:NC])
                s8 = sm.tile([B, NC], F32, name=f"s8_{t}", tag="s8")
                nc.scalar.copy(s8[:], ptr_s[:])

                gm = sm.tile([B, 1], F32, name=f"gm_{t}", tag="gm")
                nc.vector.reduce_max(gm[:], v8[:], axis=mybir.AxisListType.X)
                ngm = sm.tile([B, 1], F32, name=f"ngm_{t}", tag="ngm")
                nc.vector.tensor_scalar_mul(ngm[:], gm[:], -1.0)
                e8 = sm.tile([B, NC], F32, name=f"e8_{t}", tag="e8")
                nc.scalar.activation(e8[:], v8[:], AF.Exp, bias=ngm[:, 0:1])
                S = sm.tile([B, 1], F32, name=f"S_{t}", tag="S")
                nc.vector.scalar_tensor_tensor(
                    e8[:], e8[:], 1.0, s8[:], op0=ALU.mult, op1=ALU.mult,
                    accum_out=S[:])
                nlse = sm.tile([B, 1], F32, name=f"nlse_{t}", tag="nlse", bufs=2)
                nc.scalar.activation(nlse[:], S[:], AF.Ln)
                nc.vector.scalar_tensor_tensor(
                    nlse[:], nlse[:], -1.0, gm[:], op0=ALU.mult, op1=ALU.subtract)

                if t < n_steps - 1:
                    ptr_i = pstr.tile([B, NC], F32, name=f"tri_{t}", tag="tr")
                    nc.tensor.transpose(ptr_i[:], ags[:, B:2 * B], ident[0:NC, 0:NC])
                    i8 = sm.tile([B, NC], F32, name=f"i8_{t}", tag="i8")
                    nc.scalar.copy(i8[:], ptr_i[:])
                    eq8 = sm.tile([B, NC], F32, name=f"eq8_{t}", tag="eq8")
                    nc.vector.tensor_scalar(eq8[:], v8[:], gm[:, 0:1], None,
                                            op0=ALU.is_equal)
                    key8 = sm.tile([B, NC], F32, name=f"key8_{t}", tag="key8")
                    nc.vector.scalar_tensor_tensor(
                        key8[:], eq8[:], -BIGIDX, i8[:], op0=ALU.mult, op1=ALU.add)
                    tokf = sm.tile([B, 1], F32, name=f"tokf_{t}", tag="tokf")
                    nc.vector.tensor_reduce(tokf[:], key8[:],
                                            axis=mybir.AxisListType.X, op=ALU.min)
                    nc.vector.tensor_scalar_add(tokf[:], tokf[:], BIGIDX)
                    tok_i = sm.tile([B, 1], I32, name=f"toki_{t}", tag="toki", bufs=2)
                    nc.vector.tensor_copy(tok_i[:], tokf[:])

                # ---- output: log_probs = stage - lse (4 chunks) ----
                qt = VC // 4
                for ch in range(4):
                    a, b2 = ch * qt, (ch + 1) * qt
                    outt = stg.tile([B, qt], F32, name=f"outt_{t}_{ch}", tag="outt")
                    nc.scalar.activation(outt[:], stage[:, a:b2], AF.Identity,
                                         bias=nlse[:, 0:1])
                    nc.sync.dma_start(logp_d[t, :, a:b2], outt[:])

            nc.sync.dma_start(hout_d[:], hs_prev[:])

    nc.finalize()
    return nc


# ---------------- host side ----------------

def _prep_inputs(encoder_outputs, encoder_hidden, emb, W_ih, W_hh, b_ih, b_hh,
                 W_out, b_out):
    emb = np.ascontiguousarray(emb, np.float32)
    h0 = np.ascontiguousarray(encoder_hidden[0], np.float32)      # [B, H]
    h0T = np.ascontiguousarray(h0.T)                               # [H, B]
    x0 = np.maximum(emb[0], 0.0)                                   # [H]
    x0T = np.repeat(x0[:, None], B, 1)                             # [H, B]

    def ktile(a):  # [H, F] -> [128, KT, F]
        F = a.shape[1]
        return np.ascontiguousarray(a.reshape(KT, 128, F).transpose(1, 0, 2))

    in_maps = []
    for c in range(NC):
        gsl = np.r_[128 * c:128 * (c + 1),
                    H + 128 * c:H + 128 * (c + 1),
                    2 * H + 128 * c:2 * H + 128 * (c + 1)]
        Wih_c = W_ih[gsl]                                          # [384, H]
        Whh_c = W_hh[gsl]
        gxb = np.concatenate([
            b_ih[gsl[:256]] + b_hh[gsl[:256]], b_ih[gsl[256:]]])   # [384]
        ghb = np.concatenate([np.zeros(256, np.float32), b_hh[gsl[256:]]])
        Wout_c = W_out[VC * c:VC * (c + 1)]                        # [4000, H]
        bout8 = np.ascontiguousarray(b_out[VC * c:VC * (c + 1)], np.float32)[None, :]
        iota = np.zeros((B, VC // 4), np.float32)
        iota[:] = (VC * c + np.arange(VC // 4, dtype=np.float32))[None, :]
        qoff = np.zeros((B, 4), np.float32)
        qoff[:] = (np.arange(4, dtype=np.float32) * (VC // 4))[None, :]
        in_maps.append({
            "wih": ktile(Wih_c.T.astype(np.float32)),
            "whh": ktile(Whh_c.T.astype(np.float32)),
            "gxb": gxb.astype(np.float32)[None, :],
            "ghb": ghb.astype(np.float32)[None, :],
            "wout": ktile(Wout_c.T.astype(np.float32)),
            "bout": bout8,
            "emb": emb,
            "h0t": ktile(h0T),
            "h0s": np.ascontiguousarray(h0[:, 128 * c:128 * (c + 1)]),
            "x0t": ktile(x0T.astype(np.float32)),
            "iota": iota,
            "qoff": qoff,
            "ident": np.eye(32, dtype=np.float32),
            "ones": np.ones((1, B), np.float32),
        })
    return in_maps


_CACHE = {}


def kernel(**inputs):
    inputs = {k: np.asarray(v) for k, v in inputs.items()}
    in_maps = _prep_inputs(**inputs)
    if "nc" not in _CACHE:
        _CACHE["nc"] = build_kernel()
    from concourse.bass_utils import run_bass_kernel_spmd
    res = run_bass_kernel_spmd(_CACHE["nc"], in_maps, core_ids=list(range(NC)))
    logp = np.concatenate([res.results[c]["logp"] for c in range(NC)], axis=-1)
    logp = np.ascontiguousarray(np.moveaxis(logp, 0, 1))           # [B, T, V]
    hout = np.concatenate([res.results[c]["hout"] for c in range(NC)], axis=-1)
    return logp, hout[None]
